# revision 47
# baseline (speedup 1.0000x reference)
"""Trainium2 Bass kernel for nn_Attention_26027501814371 (gnn_message_passing).

Reference computation (K=32, B=16384, DA=128):
    w = softmax(neigh_effect, axis=0)                       # [K, B]
    t = neigh_transform.reshape(K, DA, B) * w[:, None, :]   # reshape trick!
    x_sum = t.reshape(K, B, DA).sum(axis=0)                 # [B, DA]
    x_ = n_param * x_sum
    effect = sigmoid(x_ @ ew + eb)[:, None]                 # [B, 1]
    transform = tw * x_ + tb
    return effect, effect * transform

The reshape trick means the weight applied to neigh_transform[k, b, d] is
w[k, (b % 128) * 128 + d]  (since B = 128*DA and DA = 128).  So every
128-row tile of B (starting at a multiple of 128) uses the same [128,128]
weight tile V[k] = w[k].reshape(128, 128).

Strategy: pure data-parallel over B across 8 cores (2048 rows each, 16
tiles of 128 rows).  neigh_effect is replicated (softmax recomputed per
core, it is tiny).  Each core's 32 MiB neigh_transform shard is passed
as FOUR contiguous quarter tensors (host-side slicing) so that within a
quarter the (k, tile) dims merge into a single stride-16384 AP dim —
each 4 MiB k-chunk DMA is a clean 3-dim pattern at near-peak HBM BW.
Per quarter: 2 chunk DMAs, DVE multiplies by the prebuilt
V2[r, (k,d)] = softmax * n_param (broadcast across tiles), TensorE
reduces over k with 32 wide [128, 512] identity matmuls accumulating
into one PSUM bank, then a quarter-wide vectorized epilogue computes
effect = sigmoid(x @ ew + eb) and effect * (tw * x + tb) and stores the
quarter with one DMA — overlapping the next quarter's DMA stream, so
only the last quarter's epilogue sits in the pipeline tail.
"""

import os
import sys

import numpy as np

for _p in ("/opt/trn_rl_repo",):
    if _p not in sys.path and os.path.isdir(_p):
        sys.path.insert(0, _p)

K, B, DA = 32, 16384, 128
N_CORES = 8
B_LOCAL = B // N_CORES  # 2048
P = 128  # partitions / tile rows

# set by test.py for profiling; harness default is no tracing
TRACE = False
LAST_EXEC_NS = None
LAST_TRACE_DIR = None
LAST_RESULTS = None

_BUILD_CACHE = {}

# products + identity in bf16: halves TensorE weight-load cost (fast weight
# load needs non-fp32), PSUM accumulation stays fp32.  Adds ~0.2-0.4% error
# from rounding each product to bf16 — far under the 2e-2 gate.
USE_BF16_P = False
# also cast nt2/v2 inputs of the big product pass to bf16 (ACT-engine copy),
# unlocking the DVE 2x bf16 tensor_tensor mode for the dominant pass.
USE_BF16_NT = False


def build(b_local=B_LOCAL, repeat=1, stage="full"):
    """Build the Bass graph. repeat>1 wraps the whole body in a hardware
    loop re-running the identical computation (used only for timing: the
    per-iteration delta isolates on-device exec time from tunnel/dispatch
    overhead). stage ("dma"|"prod"|"mm"|"full") truncates the pipeline for
    perf bisection."""
    import contextlib

    import concourse.mybir as mybir
    from concourse import bacc
    from concourse.tile import TileContext

    f32 = mybir.dt.float32
    bf16 = mybir.dt.bfloat16
    p_dt = bf16 if USE_BF16_P else f32
    add = mybir.AluOpType.add
    mult = mybir.AluOpType.mult
    n_tiles = b_local // P

    nc = bacc.Bacc()

    # shard passed as contiguous quarter tensors: within each quarter the
    # (k, t) dims merge into one stride-16384 AP dim (3-dim DMA), AND each
    # quarter's PSUM accumulation finishes early so its epilogue overlaps
    # the next quarter's DMA stream.
    QT = min(4, n_tiles)            # tiles per quarter / PSUM bank group
    n_q = n_tiles // QT
    nt_exts = [
        nc.declare_dram_parameter(f"nt{q}", [K, QT * P, DA], f32, isOutput=False)
        for q in range(n_q)
    ]
    ne_ext = nc.declare_dram_parameter("neigh_effect", [K, B], f32, isOutput=False)
    # packed [ident | npb | ewb | twb | tbb | ebb] -> [128, 641], one DMA
    cs_ext = nc.declare_dram_parameter("consts", [P, 5 * P + 1], f32, isOutput=False)
    eff_ext = nc.declare_dram_parameter("effect", [b_local, 1], f32, isOutput=True)
    out2_ext = nc.declare_dram_parameter("out2", [b_local, DA], f32, isOutput=True)
    CH = 2  # tiles per DMA chunk
    n_chunks = n_tiles // CH

    with TileContext(nc) as tc:
        with (
            tc.tile_pool(name="const", bufs=1) as constp,
            tc.tile_pool(name="soft", bufs=1) as softp,
            tc.tile_pool(name="nt", bufs=2) as ntp,
            tc.tile_pool(name="prod", bufs=2) as prodp,
            tc.tile_pool(name="small", bufs=2) as smallp,
            tc.tile_pool(name="psum", bufs=4, space="PSUM") as psump,
        ):
          loop_cm = (
              tc.For_i(0, repeat, 1, hint_engines=(mybir.EngineType.PE,))
              if repeat > 1
              else contextlib.nullcontext()
          )
          with loop_cm:
            # --- constants (replicated + packed host-side) ---
            cs = constp.tile([P, 5 * P + 1], f32)
            nc.sync.dma_start(out=cs[:], in_=cs_ext[:])
            ident_f = cs[:, 0:P]
            npb = cs[:, P : 2 * P]
            ewb = cs[:, 2 * P : 3 * P]
            twb = cs[:, 3 * P : 4 * P]
            tbb = cs[:, 4 * P : 5 * P]
            ebb = cs[:, 5 * P : 5 * P + 1]
            if USE_BF16_P:
                identb = constp.tile([P, P], bf16)
                nc.vector.tensor_copy(identb[:], ident_f)
                ident = identb[:]
            else:
                ident = ident_f
            eff_all = constp.tile([P, n_tiles], f32)

            # --- softmax prologue: V2[r, k*128+d] = w[k, r*128+d] * n_param[d]
            # neigh_effect values are uniform[0,1), so exp() without the max
            # subtraction is numerically safe (matches jax softmax to fp eps).
            ne2 = softp.tile([P, K * DA], f32)
            nc.sync.dma_start(
                out=ne2[:].rearrange("r (k d) -> r k d", k=K),
                in_=ne_ext[:].rearrange("k (r d) -> r k d", r=P),
            )
            ex = softp.tile([P, K * DA], f32)
            nc.scalar.activation(
                out=ex[:], in_=ne2[:], func=mybir.ActivationFunctionType.Exp
            )
            denom = smallp.tile([P, DA], f32)
            nc.vector.tensor_reduce(
                out=denom[:],
                in_=ex[:].rearrange("r (k d) -> r d k", k=K),
                axis=mybir.AxisListType.X,
                op=add,
            )
            rden = smallp.tile([P, DA], f32)
            nc.vector.reciprocal(rden[:], denom[:])
            rden2 = smallp.tile([P, DA], f32)
            nc.vector.tensor_mul(rden2[:], rden[:], npb)
            v2 = softp.tile([P, K * DA], f32)
            nc.vector.tensor_tensor(
                out=v2[:].rearrange("r (k d) -> r k d", k=K),
                in0=ex[:].rearrange("r (k d) -> r k d", k=K),
                in1=rden2[:].unsqueeze(1).broadcast_to((P, K, DA)),
                op=mult,
            )
            if USE_BF16_NT:
                v2b = softp.tile([P, K * DA], bf16)
                nc.scalar.activation(
                    out=v2b[:], in_=v2[:], func=mybir.ActivationFunctionType.Copy
                )
                v2 = v2b

            # --- quarter-wise main loop: per quarter, 2 k-chunk DMAs
            # (4 MiB, 3-dim merged AP), products, 32 wide [128,512] matmuls
            # into one PSUM bank, then the quarter epilogue — which overlaps
            # the next quarter's DMA stream.
            import concourse.bass as _bass

            KC = (2 * K * P) // (QT * P)     # k per chunk so chunk = 32KB/part
            n_kchunks = K // KC
            kt_stride = P * DA               # 16384

            for q in range(n_q):
                xg = psump.tile([P, QT * P], f32, tag="xg", name=f"xg{q}")
                for kc in range(n_kchunks):
                    nt2 = ntp.tile([P, KC * QT * DA], f32)
                    nc.sync.dma_start(
                        out=nt2[:],
                        in_=_bass.AP(
                            nt_exts[q],
                            kc * KC * QT * P * DA,
                            [[DA, P], [kt_stride, KC * QT], [1, DA]],
                        ),
                    )
                    if stage == "dma":
                        nc.scalar.dma_start(
                            out=out2_ext[
                                (q * n_kchunks + kc) * P : (q * n_kchunks + kc + 1) * P,
                                :,
                            ],
                            in_=nt2[:, 0:DA],
                        )
                        continue
                    prod = prodp.tile([P, KC * QT * DA], p_dt)
                    nc.vector.tensor_tensor(
                        out=prod[:].rearrange("r (k t d) -> r k t d", k=KC, t=QT),
                        in0=nt2[:].rearrange("r (k t d) -> r k t d", k=KC, t=QT),
                        in1=v2[:, kc * KC * DA : (kc + 1) * KC * DA]
                        .rearrange("r (k d) -> r k d", k=KC)
                        .unsqueeze(2)
                        .broadcast_to((P, KC, QT, DA)),
                        op=mult,
                    )
                    for kk in range(KC):
                        nc.tensor.matmul(
                            xg[:],
                            ident,
                            prod[:, kk * QT * DA : (kk + 1) * QT * DA],
                            start=(kc == 0 and kk == 0),
                            stop=(kc == n_kchunks - 1 and kk == KC - 1),
                        )
                if stage == "dma":
                    continue
                # --- quarter epilogue: [128, QT*128] wide vectorized ops ---
                ew_b = ewb.unsqueeze(1).broadcast_to((P, QT, DA))
                tw_b = twb.unsqueeze(1).broadcast_to((P, QT, DA))
                tb_b = tbb.unsqueeze(1).broadcast_to((P, QT, DA))
                x3 = xg[:].rearrange("r (t d) -> r t d", t=QT)
                escr = smallp.tile([P, QT * DA], f32, tag="escr")
                nc.vector.tensor_tensor(
                    out=escr[:].rearrange("r (t d) -> r t d", t=QT),
                    in0=x3, in1=ew_b, op=mult,
                )
                epg = smallp.tile([P, QT], f32, tag="epg")
                nc.vector.tensor_reduce(
                    out=epg[:],
                    in_=escr[:].rearrange("r (t d) -> r t d", t=QT),
                    axis=mybir.AxisListType.X,
                    op=add,
                )
                # z/z3 don't need eff — emit them before the sigmoid so the
                # ACT round-trip latency hides behind them on the DVE queue
                z = smallp.tile([P, QT * DA], f32, tag="zg")
                z3 = z[:].rearrange("r (t d) -> r t d", t=QT)
                nc.vector.tensor_tensor(out=z3, in0=x3, in1=tw_b, op=mult)
                nc.vector.tensor_tensor(out=z3, in0=z3, in1=tb_b, op=add)
                eff_cols = eff_all[:, q * QT : (q + 1) * QT]
                nc.scalar.activation(
                    out=eff_cols,
                    in_=epg[:],
                    func=mybir.ActivationFunctionType.Sigmoid,
                    bias=ebb,
                    scale=1.0,
                )
                eff_b = eff_cols.unsqueeze(2).broadcast_to((P, QT, DA))
                o2 = smallp.tile([P, QT * DA], f32, tag="o2g")
                o23 = o2[:].rearrange("r (t d) -> r t d", t=QT)
                nc.vector.tensor_tensor(out=o23, in0=z3, in1=eff_b, op=mult)
                nc.scalar.dma_start(
                    out=out2_ext[q * QT * P : (q + 1) * QT * P, :].rearrange(
                        "(t r) d -> r t d", r=P
                    ),
                    in_=o23,
                )

            # effect[t*128 + r] = eff_all[r, t]
            if stage == "full":
                nc.scalar.dma_start(
                    out=eff_ext[:].rearrange("(t r) o -> r t o", r=P),
                    in_=eff_all[:].unsqueeze(2),
                )

    nc.finalize()
    return nc


def _get_nc(b_local=B_LOCAL, repeat=1, stage="full"):
    key = (b_local, repeat, stage)
    if key not in _BUILD_CACHE:
        _BUILD_CACHE[key] = build(b_local, repeat, stage)
    return _BUILD_CACHE[key]


def prepare_in_maps(inputs):
    ne = np.ascontiguousarray(np.asarray(inputs["neigh_effect"], dtype=np.float32))
    nt = np.asarray(inputs["neigh_transform"], dtype=np.float32)
    n_param = np.asarray(inputs["n_param"], dtype=np.float32).reshape(-1)
    ew = np.asarray(inputs["ew"], dtype=np.float32).reshape(-1)
    eb = np.asarray(inputs["eb"], dtype=np.float32).reshape(-1)
    tw = np.asarray(inputs["tw"], dtype=np.float32).reshape(-1)
    tb = np.asarray(inputs["tb"], dtype=np.float32).reshape(-1)

    consts = np.concatenate(
        [
            np.eye(P, dtype=np.float32),
            np.tile(n_param[None, :], (P, 1)),
            np.tile(ew[None, :], (P, 1)),
            np.tile(tw[None, :], (P, 1)),
            np.tile(tb[None, :], (P, 1)),
            np.full((P, 1), eb[0], dtype=np.float32),
        ],
        axis=1,
    ).astype(np.float32)

    n_q = 4
    q_rows = B_LOCAL // n_q  # 512
    in_maps = []
    for c in range(N_CORES):
        m = {"neigh_effect": ne, "consts": consts}
        for q in range(n_q):
            lo = c * B_LOCAL + q * q_rows
            m[f"nt{q}"] = np.ascontiguousarray(nt[:, lo : lo + q_rows, :])
        in_maps.append(m)
    return in_maps


_RUNNER_CACHE = {}


def _make_runner(nc):
    """Mirror of bass2jax.run_bass_via_pjrt's multi-core path, but returning
    a reusable callable so repeated invocations hit the jax.jit cache
    instead of recompiling."""
    import jax
    from concourse import bass2jax, mybir

    bass2jax.install_neuronx_cc_hook()
    partition_name = nc.partition_id_tensor.name if nc.partition_id_tensor else None
    in_names, out_names, out_avals, zero_shapes = [], [], [], []
    for alloc in nc.m.functions[0].allocations:
        if not isinstance(alloc, mybir.MemoryLocationSet):
            continue
        name = alloc.memorylocations[0].name
        if alloc.kind == "ExternalInput":
            if name != partition_name:
                in_names.append(name)
        elif alloc.kind == "ExternalOutput":
            shape = tuple(alloc.tensor_shape)
            dtype = mybir.dt.np(alloc.dtype)
            out_names.append(name)
            out_avals.append(jax.core.ShapedArray(shape, dtype))
            zero_shapes.append((shape, dtype))
    n_params = len(in_names)
    n_outs = len(out_avals)
    all_in_names = list(in_names) + list(out_names)
    if partition_name is not None:
        all_in_names.append(partition_name)

    def _body(*args):
        operands = list(args)
        if partition_name is not None:
            operands.append(bass2jax.partition_id_tensor())
        outs = bass2jax._bass_exec_p.bind(
            *operands,
            out_avals=tuple(out_avals),
            in_names=tuple(all_in_names),
            out_names=tuple(out_names),
            lowering_input_output_aliases=(),
            sim_require_finite=True,
            sim_require_nnan=True,
            nc=nc,
        )
        return tuple(outs)

    devices = jax.devices()[:N_CORES]
    mesh = bass2jax.Mesh(np.asarray(devices), ("core",))
    in_specs = (bass2jax.PartitionSpec("core"),) * (n_params + n_outs)
    out_specs = (bass2jax.PartitionSpec("core"),) * n_outs
    donate = tuple(range(n_params, n_params + n_outs))
    sharded = jax.jit(
        bass2jax.shard_map(
            _body, mesh=mesh, in_specs=in_specs, out_specs=out_specs, check_rep=False
        ),
        donate_argnums=donate,
        keep_unused=True,
    )

    def run(in_maps, device_arrays=None):
        if device_arrays is None:
            sharding = jax.sharding.NamedSharding(mesh, bass2jax.PartitionSpec("core"))
            device_arrays = [
                jax.device_put(
                    np.concatenate(
                        [np.asarray(in_maps[c][name]) for c in range(N_CORES)], axis=0
                    ),
                    sharding,
                )
                for name in in_names
            ]
        concat_zeros = [
            np.zeros((N_CORES * s[0], *s[1:]), dt) for s, dt in zero_shapes
        ]
        out_arrs = sharded(*device_arrays, *concat_zeros)
        out_arrs = [np.asarray(a) for a in out_arrs]
        return [
            {
                name: out_arrs[i].reshape(N_CORES, *out_avals[i].shape)[c]
                for i, name in enumerate(out_names)
            }
            for c in range(N_CORES)
        ], device_arrays

    run.in_names = in_names
    return run


def run_spmd(in_maps, repeat=1, device_arrays=None, stage="full"):
    key = (B_LOCAL, repeat, stage)
    if key not in _RUNNER_CACHE:
        _RUNNER_CACHE[key] = _make_runner(_get_nc(B_LOCAL, repeat, stage))
    return _RUNNER_CACHE[key](in_maps, device_arrays)


def kernel(**inputs):
    global LAST_RESULTS
    in_maps = prepare_in_maps(inputs)
    results, _ = run_spmd(in_maps)
    LAST_RESULTS = results
    effect = np.concatenate([results[i]["effect"] for i in range(N_CORES)], axis=0)
    out2 = np.concatenate([results[i]["out2"] for i in range(N_CORES)], axis=0)
    return effect, out2


# revision 50
# speedup vs baseline: 1.3885x; 1.3885x over previous
"""Trainium2 Bass kernel for nn_Attention_26027501814371 (gnn_message_passing).

Reference computation (K=32, B=16384, DA=128):
    w = softmax(neigh_effect, axis=0)                       # [K, B]
    t = neigh_transform.reshape(K, DA, B) * w[:, None, :]   # reshape trick!
    x_sum = t.reshape(K, B, DA).sum(axis=0)                 # [B, DA]
    x_ = n_param * x_sum
    effect = sigmoid(x_ @ ew + eb)[:, None]                 # [B, 1]
    transform = tw * x_ + tb
    return effect, effect * transform

The reshape trick means the weight applied to neigh_transform[k, b, d] is
w[k, (b % 128) * 128 + d]  (since B = 128*DA and DA = 128).  So every
128-row tile of B (starting at a multiple of 128) uses the same [128,128]
weight tile V[k] = w[k].reshape(128, 128).

Strategy: pure data-parallel over B across 8 cores (2048 rows each, 16
tiles of 128 rows).  neigh_effect is replicated (softmax recomputed per
core, it is tiny).  Per core, the 32 MiB neigh_transform shard is loaded
in 8 k-chunks of 4 MiB whose (k,t) dims merge into a single stride-16384
AP dim (3-dim DMA, near-peak HBM BW with few per-DMA latencies).  DVE
multiplies each chunk by the prebuilt V2[r, (k,d)] = softmax * n_param
(broadcast across tiles), and TensorE reduces over k with wide
[128, 512] identity matmuls accumulating into 4 persistent PSUM banks
(one per 4-tile group).  A group-wide vectorized epilogue computes
effect = sigmoid(x @ ew + eb) and effect * (tw * x + tb) and stores
each 4-tile group with one DMA.
"""

import os
import sys

import numpy as np

for _p in ("/opt/trn_rl_repo",):
    if _p not in sys.path and os.path.isdir(_p):
        sys.path.insert(0, _p)

K, B, DA = 32, 16384, 128
N_CORES = 8
B_LOCAL = B // N_CORES  # 2048
P = 128  # partitions / tile rows

# set by test.py for profiling; harness default is no tracing
TRACE = False
LAST_EXEC_NS = None
LAST_TRACE_DIR = None
LAST_RESULTS = None

_BUILD_CACHE = {}

# products + identity in bf16: halves TensorE weight-load cost (fast weight
# load needs non-fp32), PSUM accumulation stays fp32.  Adds ~0.2-0.4% error
# from rounding each product to bf16 — far under the 2e-2 gate.
USE_BF16_P = False
# also cast nt2/v2 inputs of the big product pass to bf16 (ACT-engine copy),
# unlocking the DVE 2x bf16 tensor_tensor mode for the dominant pass.
USE_BF16_NT = False


def build(b_local=B_LOCAL, repeat=1, stage="full"):
    """Build the Bass graph. repeat>1 wraps the whole body in a hardware
    loop re-running the identical computation (used only for timing: the
    per-iteration delta isolates on-device exec time from tunnel/dispatch
    overhead). stage ("dma"|"prod"|"mm"|"full") truncates the pipeline for
    perf bisection."""
    import contextlib

    import concourse.mybir as mybir
    from concourse import bacc
    from concourse.tile import TileContext

    f32 = mybir.dt.float32
    bf16 = mybir.dt.bfloat16
    p_dt = bf16 if USE_BF16_P else f32
    add = mybir.AluOpType.add
    mult = mybir.AluOpType.mult
    n_tiles = b_local // P

    nc = bacc.Bacc()

    nt_ext = nc.declare_dram_parameter(
        "neigh_transform", [K, b_local, DA], f32, isOutput=False
    )
    ne_ext = nc.declare_dram_parameter("neigh_effect", [K, B], f32, isOutput=False)
    # packed [ident | npb | ewb | twb | tbb | ebb] -> [128, 641], one DMA
    cs_ext = nc.declare_dram_parameter("consts", [P, 5 * P + 1], f32, isOutput=False)
    eff_ext = nc.declare_dram_parameter("effect", [b_local, 1], f32, isOutput=True)
    out2_ext = nc.declare_dram_parameter("out2", [b_local, DA], f32, isOutput=True)
    CH = 2  # tiles per DMA chunk
    n_chunks = n_tiles // CH

    with TileContext(nc) as tc:
        with (
            tc.tile_pool(name="const", bufs=1) as constp,
            tc.tile_pool(name="soft", bufs=1) as softp,
            tc.tile_pool(name="nt", bufs=3) as ntp,
            tc.tile_pool(name="prod", bufs=2) as prodp,
            tc.tile_pool(name="small", bufs=2) as smallp,
            tc.tile_pool(name="psum", bufs=4, space="PSUM") as psump,
        ):
          loop_cm = (
              tc.For_i(0, repeat, 1, hint_engines=(mybir.EngineType.PE,))
              if repeat > 1
              else contextlib.nullcontext()
          )
          with loop_cm:
            # --- constants (replicated + packed host-side) ---
            cs = constp.tile([P, 5 * P + 1], f32)
            nc.sync.dma_start(out=cs[:], in_=cs_ext[:])
            ident_f = cs[:, 0:P]
            npb = cs[:, P : 2 * P]
            ewb = cs[:, 2 * P : 3 * P]
            twb = cs[:, 3 * P : 4 * P]
            tbb = cs[:, 4 * P : 5 * P]
            ebb = cs[:, 5 * P : 5 * P + 1]
            if USE_BF16_P:
                identb = constp.tile([P, P], bf16)
                nc.vector.tensor_copy(identb[:], ident_f)
                ident = identb[:]
            else:
                ident = ident_f
            eff_all = constp.tile([P, n_tiles], f32)

            # --- softmax prologue: V2[r, k*128+d] = w[k, r*128+d] * n_param[d]
            # neigh_effect values are uniform[0,1), so exp() without the max
            # subtraction is numerically safe (matches jax softmax to fp eps).
            # ne2/ex borrow nt-pool slots (tag "nt2") so their SBUF is
            # reclaimed for a third in-flight DMA chunk after the prologue —
            # extra DMA run-ahead decouples the queue from DVE jitter.
            ne2 = ntp.tile([P, K * DA], f32, tag="nt2", name="ne2")
            nc.sync.dma_start(
                out=ne2[:].rearrange("r (k d) -> r k d", k=K),
                in_=ne_ext[:].rearrange("k (r d) -> r k d", r=P),
            )
            ex = ntp.tile([P, K * DA], f32, tag="nt2", name="ex")
            nc.scalar.activation(
                out=ex[:], in_=ne2[:], func=mybir.ActivationFunctionType.Exp
            )
            denom = smallp.tile([P, DA], f32)
            nc.vector.tensor_reduce(
                out=denom[:],
                in_=ex[:].rearrange("r (k d) -> r d k", k=K),
                axis=mybir.AxisListType.X,
                op=add,
            )
            rden = smallp.tile([P, DA], f32)
            nc.vector.reciprocal(rden[:], denom[:])
            rden2 = smallp.tile([P, DA], f32)
            nc.vector.tensor_mul(rden2[:], rden[:], npb)
            v2 = softp.tile([P, K * DA], f32)
            nc.vector.tensor_tensor(
                out=v2[:].rearrange("r (k d) -> r k d", k=K),
                in0=ex[:].rearrange("r (k d) -> r k d", k=K),
                in1=rden2[:].unsqueeze(1).broadcast_to((P, K, DA)),
                op=mult,
            )
            if USE_BF16_NT:
                v2b = softp.tile([P, K * DA], bf16)
                nc.scalar.activation(
                    out=v2b[:], in_=v2[:], func=mybir.ActivationFunctionType.Copy
                )
                v2 = v2b

            # --- k-chunked main loop: each DMA loads KC k-slices across
            # ALL tiles (4 MiB, 3D-balanced AP); wide [128, TG*128] matmuls
            # accumulate into TG-tile PSUM banks across all k-chunks; a
            # group-wide vectorized epilogue finishes each PSUM bank.
            import concourse.bass as _bass

            T = n_tiles
            KC = 4                      # k per load chunk
            n_kchunks = K // KC
            TG = min(4, T)              # tiles per PSUM bank group
            n_groups = T // TG
            kt_stride = P * DA          # 16384: stride of merged (k', t) dim

            xgs = [
                psump.tile([P, TG * P], f32, tag="xg", name=f"xg{g}")
                for g in range(n_groups)
            ]
            for kc in range(n_kchunks):
                nt2 = ntp.tile([P, KC * T * DA], f32)
                # src AP: partition r (stride 128), merged (k', t) dim
                # (stride 16384, num KC*T), d (stride 1).  offset = k0 rows.
                nc.sync.dma_start(
                    out=nt2[:],
                    in_=_bass.AP(
                        nt_ext,
                        kc * KC * b_local * DA,
                        [[DA, P], [kt_stride, KC * T], [1, DA]],
                    ),
                )
                if stage == "dma":
                    nc.scalar.dma_start(
                        out=out2_ext[kc * P : (kc + 1) * P, :], in_=nt2[:, 0:DA]
                    )
                    continue
                prod = prodp.tile([P, KC * T * DA], p_dt)
                nc.vector.tensor_tensor(
                    out=prod[:].rearrange("r (k t d) -> r k t d", k=KC, t=T),
                    in0=nt2[:].rearrange("r (k t d) -> r k t d", k=KC, t=T),
                    in1=v2[:, kc * KC * DA : (kc + 1) * KC * DA]
                    .rearrange("r (k d) -> r k d", k=KC)
                    .unsqueeze(2)
                    .broadcast_to((P, KC, T, DA)),
                    op=mult,
                )
                for kk in range(KC):
                    for g in range(n_groups):
                        nc.tensor.matmul(
                            xgs[g][:],
                            ident,
                            prod[
                                :,
                                (kk * T + g * TG) * DA : (kk * T + g * TG + TG) * DA,
                            ],
                            start=(kc == 0 and kk == 0),
                            stop=(kc == n_kchunks - 1 and kk == KC - 1),
                        )

            if stage != "dma":
                # --- epilogue per PSUM bank group: [128, TG*128] wide ops ---
                for g in range(n_groups):
                    xg = xgs[g]
                    ew_b = ewb.unsqueeze(1).broadcast_to((P, TG, DA))
                    tw_b = twb.unsqueeze(1).broadcast_to((P, TG, DA))
                    tb_b = tbb.unsqueeze(1).broadcast_to((P, TG, DA))
                    x3 = xg[:].rearrange("r (t d) -> r t d", t=TG)
                    escr = smallp.tile([P, TG * DA], f32, tag="escr")
                    nc.vector.tensor_tensor(
                        out=escr[:].rearrange("r (t d) -> r t d", t=TG),
                        in0=x3, in1=ew_b, op=mult,
                    )
                    epg = smallp.tile([P, TG], f32, tag="epg")
                    nc.vector.tensor_reduce(
                        out=epg[:],
                        in_=escr[:].rearrange("r (t d) -> r t d", t=TG),
                        axis=mybir.AxisListType.X,
                        op=add,
                    )
                    eff_cols = eff_all[:, g * TG : (g + 1) * TG]
                    nc.scalar.activation(
                        out=eff_cols,
                        in_=epg[:],
                        func=mybir.ActivationFunctionType.Sigmoid,
                        bias=ebb,
                        scale=1.0,
                    )
                    eff_b = eff_cols.unsqueeze(2).broadcast_to((P, TG, DA))
                    z = smallp.tile([P, TG * DA], f32, tag="zg")
                    z3 = z[:].rearrange("r (t d) -> r t d", t=TG)
                    nc.vector.tensor_tensor(out=z3, in0=x3, in1=tw_b, op=mult)
                    nc.vector.tensor_tensor(out=z3, in0=z3, in1=tb_b, op=add)
                    o2 = smallp.tile([P, TG * DA], f32, tag="o2g")
                    o23 = o2[:].rearrange("r (t d) -> r t d", t=TG)
                    nc.vector.tensor_tensor(out=o23, in0=z3, in1=eff_b, op=mult)
                    nc.scalar.dma_start(
                        out=out2_ext[g * TG * P : (g + 1) * TG * P, :].rearrange(
                            "(t r) d -> r t d", r=P
                        ),
                        in_=o23,
                    )

            # effect[t*128 + r] = eff_all[r, t]
            if stage == "full":
                nc.scalar.dma_start(
                    out=eff_ext[:].rearrange("(t r) o -> r t o", r=P),
                    in_=eff_all[:].unsqueeze(2),
                )

    nc.finalize()
    return nc


def _get_nc(b_local=B_LOCAL, repeat=1, stage="full"):
    key = (b_local, repeat, stage)
    if key not in _BUILD_CACHE:
        _BUILD_CACHE[key] = build(b_local, repeat, stage)
    return _BUILD_CACHE[key]


def prepare_in_maps(inputs):
    ne = np.ascontiguousarray(np.asarray(inputs["neigh_effect"], dtype=np.float32))
    nt = np.asarray(inputs["neigh_transform"], dtype=np.float32)
    n_param = np.asarray(inputs["n_param"], dtype=np.float32).reshape(-1)
    ew = np.asarray(inputs["ew"], dtype=np.float32).reshape(-1)
    eb = np.asarray(inputs["eb"], dtype=np.float32).reshape(-1)
    tw = np.asarray(inputs["tw"], dtype=np.float32).reshape(-1)
    tb = np.asarray(inputs["tb"], dtype=np.float32).reshape(-1)

    consts = np.concatenate(
        [
            np.eye(P, dtype=np.float32),
            np.tile(n_param[None, :], (P, 1)),
            np.tile(ew[None, :], (P, 1)),
            np.tile(tw[None, :], (P, 1)),
            np.tile(tb[None, :], (P, 1)),
            np.full((P, 1), eb[0], dtype=np.float32),
        ],
        axis=1,
    ).astype(np.float32)

    in_maps = []
    for c in range(N_CORES):
        in_maps.append(
            {
                "neigh_transform": np.ascontiguousarray(
                    nt[:, c * B_LOCAL : (c + 1) * B_LOCAL, :]
                ),
                "neigh_effect": ne,
                "consts": consts,
            }
        )
    return in_maps


_RUNNER_CACHE = {}


def _make_runner(nc):
    """Mirror of bass2jax.run_bass_via_pjrt's multi-core path, but returning
    a reusable callable so repeated invocations hit the jax.jit cache
    instead of recompiling."""
    import jax
    from concourse import bass2jax, mybir

    bass2jax.install_neuronx_cc_hook()
    partition_name = nc.partition_id_tensor.name if nc.partition_id_tensor else None
    in_names, out_names, out_avals, zero_shapes = [], [], [], []
    for alloc in nc.m.functions[0].allocations:
        if not isinstance(alloc, mybir.MemoryLocationSet):
            continue
        name = alloc.memorylocations[0].name
        if alloc.kind == "ExternalInput":
            if name != partition_name:
                in_names.append(name)
        elif alloc.kind == "ExternalOutput":
            shape = tuple(alloc.tensor_shape)
            dtype = mybir.dt.np(alloc.dtype)
            out_names.append(name)
            out_avals.append(jax.core.ShapedArray(shape, dtype))
            zero_shapes.append((shape, dtype))
    n_params = len(in_names)
    n_outs = len(out_avals)
    all_in_names = list(in_names) + list(out_names)
    if partition_name is not None:
        all_in_names.append(partition_name)

    def _body(*args):
        operands = list(args)
        if partition_name is not None:
            operands.append(bass2jax.partition_id_tensor())
        outs = bass2jax._bass_exec_p.bind(
            *operands,
            out_avals=tuple(out_avals),
            in_names=tuple(all_in_names),
            out_names=tuple(out_names),
            lowering_input_output_aliases=(),
            sim_require_finite=True,
            sim_require_nnan=True,
            nc=nc,
        )
        return tuple(outs)

    devices = jax.devices()[:N_CORES]
    mesh = bass2jax.Mesh(np.asarray(devices), ("core",))
    in_specs = (bass2jax.PartitionSpec("core"),) * (n_params + n_outs)
    out_specs = (bass2jax.PartitionSpec("core"),) * n_outs
    donate = tuple(range(n_params, n_params + n_outs))
    sharded = jax.jit(
        bass2jax.shard_map(
            _body, mesh=mesh, in_specs=in_specs, out_specs=out_specs, check_rep=False
        ),
        donate_argnums=donate,
        keep_unused=True,
    )

    def run(in_maps, device_arrays=None):
        if device_arrays is None:
            sharding = jax.sharding.NamedSharding(mesh, bass2jax.PartitionSpec("core"))
            device_arrays = [
                jax.device_put(
                    np.concatenate(
                        [np.asarray(in_maps[c][name]) for c in range(N_CORES)], axis=0
                    ),
                    sharding,
                )
                for name in in_names
            ]
        concat_zeros = [
            np.zeros((N_CORES * s[0], *s[1:]), dt) for s, dt in zero_shapes
        ]
        out_arrs = sharded(*device_arrays, *concat_zeros)
        out_arrs = [np.asarray(a) for a in out_arrs]
        return [
            {
                name: out_arrs[i].reshape(N_CORES, *out_avals[i].shape)[c]
                for i, name in enumerate(out_names)
            }
            for c in range(N_CORES)
        ], device_arrays

    run.in_names = in_names
    return run


def run_spmd(in_maps, repeat=1, device_arrays=None, stage="full"):
    key = (B_LOCAL, repeat, stage)
    if key not in _RUNNER_CACHE:
        _RUNNER_CACHE[key] = _make_runner(_get_nc(B_LOCAL, repeat, stage))
    return _RUNNER_CACHE[key](in_maps, device_arrays)


def kernel(**inputs):
    global LAST_RESULTS
    in_maps = prepare_in_maps(inputs)
    results, _ = run_spmd(in_maps)
    LAST_RESULTS = results
    effect = np.concatenate([results[i]["effect"] for i in range(N_CORES)], axis=0)
    out2 = np.concatenate([results[i]["out2"] for i in range(N_CORES)], axis=0)
    return effect, out2


# revision 51
# speedup vs baseline: 1.6319x; 1.1753x over previous
"""Trainium2 Bass kernel for nn_Attention_26027501814371 (gnn_message_passing).

Reference computation (K=32, B=16384, DA=128):
    w = softmax(neigh_effect, axis=0)                       # [K, B]
    t = neigh_transform.reshape(K, DA, B) * w[:, None, :]   # reshape trick!
    x_sum = t.reshape(K, B, DA).sum(axis=0)                 # [B, DA]
    x_ = n_param * x_sum
    effect = sigmoid(x_ @ ew + eb)[:, None]                 # [B, 1]
    transform = tw * x_ + tb
    return effect, effect * transform

The reshape trick means the weight applied to neigh_transform[k, b, d] is
w[k, (b % 128) * 128 + d]  (since B = 128*DA and DA = 128).  So every
128-row tile of B (starting at a multiple of 128) uses the same [128,128]
weight tile V[k] = w[k].reshape(128, 128).

Strategy: pure data-parallel over B across 8 cores (2048 rows each, 16
tiles of 128 rows).  neigh_effect is replicated (softmax recomputed per
core, it is tiny).  Per core, the 32 MiB neigh_transform shard is loaded
in 8 k-chunks of 4 MiB whose (k,t) dims merge into a single stride-16384
AP dim (3-dim DMA, near-peak HBM BW with few per-DMA latencies).  DVE
multiplies each chunk by the prebuilt V2[r, (k,d)] = softmax * n_param
(broadcast across tiles), and TensorE reduces over k with wide
[128, 512] identity matmuls accumulating into 4 persistent PSUM banks
(one per 4-tile group).  A group-wide vectorized epilogue computes
effect = sigmoid(x @ ew + eb) and effect * (tw * x + tb) and stores
each 4-tile group with one DMA.
"""

import os
import sys

import numpy as np

for _p in ("/opt/trn_rl_repo",):
    if _p not in sys.path and os.path.isdir(_p):
        sys.path.insert(0, _p)

K, B, DA = 32, 16384, 128
N_CORES = 8
B_LOCAL = B // N_CORES  # 2048
P = 128  # partitions / tile rows

# set by test.py for profiling; harness default is no tracing
TRACE = False
LAST_EXEC_NS = None
LAST_TRACE_DIR = None
LAST_RESULTS = None

_BUILD_CACHE = {}

# products + identity in bf16: halves TensorE weight-load cost (fast weight
# load needs non-fp32), PSUM accumulation stays fp32.  Adds ~0.2-0.4% error
# from rounding each product to bf16 — far under the 2e-2 gate.
USE_BF16_P = False
# also cast nt2/v2 inputs of the big product pass to bf16 (ACT-engine copy),
# unlocking the DVE 2x bf16 tensor_tensor mode for the dominant pass.
USE_BF16_NT = False


def build(b_local=B_LOCAL, repeat=1, stage="full"):
    """Build the Bass graph. repeat>1 wraps the whole body in a hardware
    loop re-running the identical computation (used only for timing: the
    per-iteration delta isolates on-device exec time from tunnel/dispatch
    overhead). stage ("dma"|"prod"|"mm"|"full") truncates the pipeline for
    perf bisection."""
    import contextlib

    import concourse.mybir as mybir
    from concourse import bacc
    from concourse.tile import TileContext

    f32 = mybir.dt.float32
    bf16 = mybir.dt.bfloat16
    p_dt = bf16 if USE_BF16_P else f32
    add = mybir.AluOpType.add
    mult = mybir.AluOpType.mult
    n_tiles = b_local // P

    nc = bacc.Bacc()

    nt_ext = nc.declare_dram_parameter(
        "neigh_transform", [K, b_local, DA], f32, isOutput=False
    )
    ne_ext = nc.declare_dram_parameter("neigh_effect", [K, B], f32, isOutput=False)
    # packed [ident | npb | ewb | twb | tbb | ebb] -> [128, 641], one DMA
    cs_ext = nc.declare_dram_parameter("consts", [P, 5 * P + 1], f32, isOutput=False)
    eff_ext = nc.declare_dram_parameter("effect", [b_local, 1], f32, isOutput=True)
    out2_ext = nc.declare_dram_parameter("out2", [b_local, DA], f32, isOutput=True)
    CH = 2  # tiles per DMA chunk
    n_chunks = n_tiles // CH

    with TileContext(nc) as tc:
        with (
            tc.tile_pool(name="const", bufs=1) as constp,
            tc.tile_pool(name="soft", bufs=1) as softp,
            tc.tile_pool(name="nt", bufs=2) as ntp,
            tc.tile_pool(name="prod", bufs=2) as prodp,
            tc.tile_pool(name="small", bufs=2) as smallp,
            tc.tile_pool(name="psum", bufs=4, space="PSUM") as psump,
        ):
          loop_cm = (
              tc.For_i(0, repeat, 1, hint_engines=(mybir.EngineType.PE,))
              if repeat > 1
              else contextlib.nullcontext()
          )
          with loop_cm:
            # --- constants (replicated + packed host-side) ---
            cs = constp.tile([P, 5 * P + 1], f32)
            nc.sync.dma_start(out=cs[:], in_=cs_ext[:])
            ident_f = cs[:, 0:P]
            npb = cs[:, P : 2 * P]
            ewb = cs[:, 2 * P : 3 * P]
            twb = cs[:, 3 * P : 4 * P]
            tbb = cs[:, 4 * P : 5 * P]
            ebb = cs[:, 5 * P : 5 * P + 1]
            if USE_BF16_P:
                identb = constp.tile([P, P], bf16)
                nc.vector.tensor_copy(identb[:], ident_f)
                ident = identb[:]
            else:
                ident = ident_f
            eff_all = constp.tile([P, n_tiles], f32)

            # --- softmax prologue: V2[r, k*128+d] = w[k, r*128+d] * n_param[d]
            # neigh_effect values are uniform[0,1), so exp() without the max
            # subtraction is numerically safe (matches jax softmax to fp eps).
            ne2 = softp.tile([P, K * DA], f32)
            nc.sync.dma_start(
                out=ne2[:].rearrange("r (k d) -> r k d", k=K),
                in_=ne_ext[:].rearrange("k (r d) -> r k d", r=P),
            )
            ex = softp.tile([P, K * DA], f32)
            nc.scalar.activation(
                out=ex[:], in_=ne2[:], func=mybir.ActivationFunctionType.Exp
            )
            denom = smallp.tile([P, DA], f32)
            nc.vector.tensor_reduce(
                out=denom[:],
                in_=ex[:].rearrange("r (k d) -> r d k", k=K),
                axis=mybir.AxisListType.X,
                op=add,
            )
            rden = smallp.tile([P, DA], f32)
            nc.vector.reciprocal(rden[:], denom[:])
            rden2 = smallp.tile([P, DA], f32)
            nc.vector.tensor_mul(rden2[:], rden[:], npb)
            v2 = softp.tile([P, K * DA], f32)
            nc.vector.tensor_tensor(
                out=v2[:].rearrange("r (k d) -> r k d", k=K),
                in0=ex[:].rearrange("r (k d) -> r k d", k=K),
                in1=rden2[:].unsqueeze(1).broadcast_to((P, K, DA)),
                op=mult,
            )
            if USE_BF16_NT:
                v2b = softp.tile([P, K * DA], bf16)
                nc.scalar.activation(
                    out=v2b[:], in_=v2[:], func=mybir.ActivationFunctionType.Copy
                )
                v2 = v2b

            # --- k-chunked main loop: each DMA loads KC k-slices across
            # ALL tiles (4 MiB, 3D-balanced AP); wide [128, TG*128] matmuls
            # accumulate into TG-tile PSUM banks across all k-chunks; a
            # group-wide vectorized epilogue finishes each PSUM bank.
            import concourse.bass as _bass

            T = n_tiles
            KC = 4                      # k per load chunk
            n_kchunks = K // KC
            TG = min(4, T)              # tiles per PSUM bank group
            n_groups = T // TG
            kt_stride = P * DA          # 16384: stride of merged (k', t) dim

            xgs = [
                psump.tile([P, TG * P], f32, tag="xg", name=f"xg{g}")
                for g in range(n_groups)
            ]
            for kc in range(n_kchunks):
                nt2 = ntp.tile([P, KC * T * DA], f32)
                # src AP: partition r (stride 128), merged (k', t) dim
                # (stride 16384, num KC*T), d (stride 1).  offset = k0 rows.
                nc.sync.dma_start(
                    out=nt2[:],
                    in_=_bass.AP(
                        nt_ext,
                        kc * KC * b_local * DA,
                        [[DA, P], [kt_stride, KC * T], [1, DA]],
                    ),
                )
                if stage == "dma":
                    nc.scalar.dma_start(
                        out=out2_ext[kc * P : (kc + 1) * P, :], in_=nt2[:, 0:DA]
                    )
                    continue
                prod = prodp.tile([P, KC * T * DA], p_dt)
                nc.vector.tensor_tensor(
                    out=prod[:].rearrange("r (k t d) -> r k t d", k=KC, t=T),
                    in0=nt2[:].rearrange("r (k t d) -> r k t d", k=KC, t=T),
                    in1=v2[:, kc * KC * DA : (kc + 1) * KC * DA]
                    .rearrange("r (k d) -> r k d", k=KC)
                    .unsqueeze(2)
                    .broadcast_to((P, KC, T, DA)),
                    op=mult,
                )
                for kk in range(KC):
                    for g in range(n_groups):
                        nc.tensor.matmul(
                            xgs[g][:],
                            ident,
                            prod[
                                :,
                                (kk * T + g * TG) * DA : (kk * T + g * TG + TG) * DA,
                            ],
                            start=(kc == 0 and kk == 0),
                            stop=(kc == n_kchunks - 1 and kk == KC - 1),
                        )

            if stage != "dma":
                # --- epilogue per PSUM bank group: [128, TG*128] wide ops ---
                for g in range(n_groups):
                    xg = xgs[g]
                    ew_b = ewb.unsqueeze(1).broadcast_to((P, TG, DA))
                    tw_b = twb.unsqueeze(1).broadcast_to((P, TG, DA))
                    tb_b = tbb.unsqueeze(1).broadcast_to((P, TG, DA))
                    x3 = xg[:].rearrange("r (t d) -> r t d", t=TG)
                    escr = smallp.tile([P, TG * DA], f32, tag="escr")
                    nc.vector.tensor_tensor(
                        out=escr[:].rearrange("r (t d) -> r t d", t=TG),
                        in0=x3, in1=ew_b, op=mult,
                    )
                    epg = smallp.tile([P, TG], f32, tag="epg")
                    nc.vector.tensor_reduce(
                        out=epg[:],
                        in_=escr[:].rearrange("r (t d) -> r t d", t=TG),
                        axis=mybir.AxisListType.X,
                        op=add,
                    )
                    eff_cols = eff_all[:, g * TG : (g + 1) * TG]
                    nc.scalar.activation(
                        out=eff_cols,
                        in_=epg[:],
                        func=mybir.ActivationFunctionType.Sigmoid,
                        bias=ebb,
                        scale=1.0,
                    )
                    eff_b = eff_cols.unsqueeze(2).broadcast_to((P, TG, DA))
                    z = smallp.tile([P, TG * DA], f32, tag="zg")
                    z3 = z[:].rearrange("r (t d) -> r t d", t=TG)
                    nc.vector.tensor_tensor(out=z3, in0=x3, in1=tw_b, op=mult)
                    nc.vector.tensor_tensor(out=z3, in0=z3, in1=tb_b, op=add)
                    o2 = smallp.tile([P, TG * DA], f32, tag="o2g")
                    o23 = o2[:].rearrange("r (t d) -> r t d", t=TG)
                    nc.vector.tensor_tensor(out=o23, in0=z3, in1=eff_b, op=mult)
                    nc.scalar.dma_start(
                        out=out2_ext[g * TG * P : (g + 1) * TG * P, :].rearrange(
                            "(t r) d -> r t d", r=P
                        ),
                        in_=o23,
                    )

            # effect[t*128 + r] = eff_all[r, t]
            if stage == "full":
                nc.scalar.dma_start(
                    out=eff_ext[:].rearrange("(t r) o -> r t o", r=P),
                    in_=eff_all[:].unsqueeze(2),
                )

    nc.finalize()
    return nc


def _get_nc(b_local=B_LOCAL, repeat=1, stage="full"):
    key = (b_local, repeat, stage)
    if key not in _BUILD_CACHE:
        _BUILD_CACHE[key] = build(b_local, repeat, stage)
    return _BUILD_CACHE[key]


def prepare_in_maps(inputs):
    ne = np.ascontiguousarray(np.asarray(inputs["neigh_effect"], dtype=np.float32))
    nt = np.asarray(inputs["neigh_transform"], dtype=np.float32)
    n_param = np.asarray(inputs["n_param"], dtype=np.float32).reshape(-1)
    ew = np.asarray(inputs["ew"], dtype=np.float32).reshape(-1)
    eb = np.asarray(inputs["eb"], dtype=np.float32).reshape(-1)
    tw = np.asarray(inputs["tw"], dtype=np.float32).reshape(-1)
    tb = np.asarray(inputs["tb"], dtype=np.float32).reshape(-1)

    consts = np.concatenate(
        [
            np.eye(P, dtype=np.float32),
            np.tile(n_param[None, :], (P, 1)),
            np.tile(ew[None, :], (P, 1)),
            np.tile(tw[None, :], (P, 1)),
            np.tile(tb[None, :], (P, 1)),
            np.full((P, 1), eb[0], dtype=np.float32),
        ],
        axis=1,
    ).astype(np.float32)

    in_maps = []
    for c in range(N_CORES):
        in_maps.append(
            {
                "neigh_transform": np.ascontiguousarray(
                    nt[:, c * B_LOCAL : (c + 1) * B_LOCAL, :]
                ),
                "neigh_effect": ne,
                "consts": consts,
            }
        )
    return in_maps


_RUNNER_CACHE = {}


def _make_runner(nc):
    """Mirror of bass2jax.run_bass_via_pjrt's multi-core path, but returning
    a reusable callable so repeated invocations hit the jax.jit cache
    instead of recompiling."""
    import jax
    from concourse import bass2jax, mybir

    bass2jax.install_neuronx_cc_hook()
    partition_name = nc.partition_id_tensor.name if nc.partition_id_tensor else None
    in_names, out_names, out_avals, zero_shapes = [], [], [], []
    for alloc in nc.m.functions[0].allocations:
        if not isinstance(alloc, mybir.MemoryLocationSet):
            continue
        name = alloc.memorylocations[0].name
        if alloc.kind == "ExternalInput":
            if name != partition_name:
                in_names.append(name)
        elif alloc.kind == "ExternalOutput":
            shape = tuple(alloc.tensor_shape)
            dtype = mybir.dt.np(alloc.dtype)
            out_names.append(name)
            out_avals.append(jax.core.ShapedArray(shape, dtype))
            zero_shapes.append((shape, dtype))
    n_params = len(in_names)
    n_outs = len(out_avals)
    all_in_names = list(in_names) + list(out_names)
    if partition_name is not None:
        all_in_names.append(partition_name)

    def _body(*args):
        operands = list(args)
        if partition_name is not None:
            operands.append(bass2jax.partition_id_tensor())
        outs = bass2jax._bass_exec_p.bind(
            *operands,
            out_avals=tuple(out_avals),
            in_names=tuple(all_in_names),
            out_names=tuple(out_names),
            lowering_input_output_aliases=(),
            sim_require_finite=True,
            sim_require_nnan=True,
            nc=nc,
        )
        return tuple(outs)

    devices = jax.devices()[:N_CORES]
    mesh = bass2jax.Mesh(np.asarray(devices), ("core",))
    in_specs = (bass2jax.PartitionSpec("core"),) * (n_params + n_outs)
    out_specs = (bass2jax.PartitionSpec("core"),) * n_outs
    donate = tuple(range(n_params, n_params + n_outs))
    sharded = jax.jit(
        bass2jax.shard_map(
            _body, mesh=mesh, in_specs=in_specs, out_specs=out_specs, check_rep=False
        ),
        donate_argnums=donate,
        keep_unused=True,
    )

    def run(in_maps, device_arrays=None):
        if device_arrays is None:
            sharding = jax.sharding.NamedSharding(mesh, bass2jax.PartitionSpec("core"))
            device_arrays = [
                jax.device_put(
                    np.concatenate(
                        [np.asarray(in_maps[c][name]) for c in range(N_CORES)], axis=0
                    ),
                    sharding,
                )
                for name in in_names
            ]
        concat_zeros = [
            np.zeros((N_CORES * s[0], *s[1:]), dt) for s, dt in zero_shapes
        ]
        out_arrs = sharded(*device_arrays, *concat_zeros)
        out_arrs = [np.asarray(a) for a in out_arrs]
        return [
            {
                name: out_arrs[i].reshape(N_CORES, *out_avals[i].shape)[c]
                for i, name in enumerate(out_names)
            }
            for c in range(N_CORES)
        ], device_arrays

    run.in_names = in_names
    return run


def run_spmd(in_maps, repeat=1, device_arrays=None, stage="full"):
    key = (B_LOCAL, repeat, stage)
    if key not in _RUNNER_CACHE:
        _RUNNER_CACHE[key] = _make_runner(_get_nc(B_LOCAL, repeat, stage))
    return _RUNNER_CACHE[key](in_maps, device_arrays)


def kernel(**inputs):
    global LAST_RESULTS
    in_maps = prepare_in_maps(inputs)
    results, _ = run_spmd(in_maps)
    LAST_RESULTS = results
    effect = np.concatenate([results[i]["effect"] for i in range(N_CORES)], axis=0)
    out2 = np.concatenate([results[i]["out2"] for i in range(N_CORES)], axis=0)
    return effect, out2
